# revision 27
# baseline (speedup 1.0000x reference)
"""Llama attention layer (B=2, S=2048, D=2048, H=16, DH=128) on 8 TRN2 NeuronCores.

Sharding: 2-way data parallel over batch x 4-way tensor parallel over heads.
Core c: batch g = c // 4, heads 4r..4r+3 where r = c % 4.
Projections are column-parallel (each core computes Q/K/V for its 4 heads),
attention is fully local per (batch, head), then the per-head attention
outputs (kept transposed, [dim, seq]) are AllGather'd within each 4-core
batch group in seq-chunks, and o_proj is column-parallel: core c computes
output columns r*512..(r+1)*512 of its batch. Host concatenates.

v3 structure:
- Projection and attention blocks are emitted interleaved in two
  superblocks (P01 A0 AG0 A1 AG1 | P23 A2 AG2 A3 AG3) so each seq-chunk's
  AllGather triggers as soon as that query block's attention finishes; the
  serial AG chain runs entirely under the remaining compute.
- Q/K sweeps process one head over BOTH seq blocks of a superblock with
  the weight tile stationary, so consecutive matmuls share one LDWEIGHTS.
  o_proj chunks (0,1) are paired the same way over one weight sweep.
- The LAST block's AllGather is split per head: each head's output is
  gathered as soon as that head finishes, so only the final head's small
  AG sits in the tail.
- PSUM budget 7 banks: [128,2,512] "main" x2 (all sweeps + score tiles),
  [128,512] "av" x2, [128,512] "den" x1.
- Attention is software-pipelined one k-tile-pair ahead (scores+exp of
  pair j+1 issue before the AV matmuls of pair j) so the ACT engine's exp
  latency stays off the TensorE critical path.

All matmul operands are bf16 (fp32 accumulation in PSUM); softmax runs
without max-subtraction; the denominator is accumulated with ones-matmuls
(partition-replicated).
"""

import os
import sys

for _p in ("/opt/trn_rl_repo", "/root/.axon_site/_ro/trn_rl_repo"):
    if os.path.isdir(_p) and _p not in sys.path:
        sys.path.append(_p)

import numpy as np
import ml_dtypes

import concourse.bass as bass
import concourse.tile as tile
import concourse.mybir as mybir
from concourse import bacc
from concourse.bass_utils import run_bass_kernel_spmd

F32 = mybir.dt.float32
BF16 = mybir.dt.bfloat16
AF = mybir.ActivationFunctionType

B, S, D, H, DH = 2, 2048, 2048, 16, 128
NCORES = 8
TP = 4                 # cores per batch group
HPC = H // TP          # heads per core = 4
SBLK = 512             # seq block (matmul moving size)
NSB = S // SBLK        # 4
DTILES = D // 128      # 16 contraction tiles
KT = S // 128          # 16 key tiles
OCOLS = D // TP        # 512 output columns per core
SCALE = 1.0 / float(np.sqrt(DH))

DT = BF16              # matmul operand dtype
NPDT = ml_dtypes.bfloat16


def _split_load(nc, dst, src_2d, nchunks=4, eng=None):
    """DMA a [D, inner] DRAM tensor into dst [128, DTILES, inner] in
    chunks along the d-tile axis."""
    eng = eng or nc.sync
    step = DTILES // nchunks
    for i in range(nchunks):
        t0 = i * step
        eng.dma_start(
            dst[:, t0:t0 + step, :],
            src_2d[t0 * 128:(t0 + step) * 128, :].rearrange(
                "(t p) s -> p t s", p=128),
        )


def _emit(tc):
    nc = tc.nc
    xT = nc.dram_tensor("xT", [D, S], DT, kind="ExternalInput").ap()
    wqT = nc.dram_tensor("wqT", [D, HPC * DH], DT, kind="ExternalInput").ap()
    wkT = nc.dram_tensor("wkT", [D, HPC * DH], DT, kind="ExternalInput").ap()
    wvT = nc.dram_tensor("wvT", [D, HPC * DH], DT, kind="ExternalInput").ap()
    woT = nc.dram_tensor("woT", [D, OCOLS], DT, kind="ExternalInput").ap()
    cosT = nc.dram_tensor("cosT", [DH, S], DT, kind="ExternalInput").ap()
    srotT = nc.dram_tensor("srotT", [DH, S], DT, kind="ExternalInput").ap()
    masks = nc.dram_tensor("masks", [4, 128, SBLK], DT, kind="ExternalInput").ap()
    outT = nc.dram_tensor("outT", [OCOLS, S], F32, kind="ExternalOutput").ap()

    # AllGather bounce buffers: whole-block chunks for blocks 0..2, per-head
    # chunks for the last block (so the tail only waits on one head's AG).
    vloc = [nc.dram_tensor(f"vals_loc_{c}", [HPC * DH, SBLK], DT).ap()
            for c in range(NSB - 1)]
    vgath = [nc.dram_tensor(f"vals_gath_{c}", [D, SBLK], DT).ap()
             for c in range(NSB - 1)]
    vloc3h = [nc.dram_tensor(f"vals_loc3_{h}", [DH, SBLK], DT).ap()
              for h in range(HPC)]
    vgath3h = [nc.dram_tensor(f"vals_gath3_{h}", [TP * DH, SBLK], DT).ap()
               for h in range(HPC)]
    wup_in = nc.dram_tensor("wup_in", [128, 4], DT).ap()
    wup_out = nc.dram_tensor("wup_out", [512, 4], DT).ap()

    with tc.tile_pool(name="const", bufs=1) as cpool, \
         tc.tile_pool(name="qkv", bufs=1) as qkvpool, \
         tc.tile_pool(name="wts", bufs=1) as wpool, \
         tc.tile_pool(name="xvg", bufs=2) as xpool, \
         tc.tile_pool(name="v3p", bufs=3) as v3pool, \
         tc.tile_pool(name="ps", bufs=2, space="PSUM") as psp, \
         tc.tile_pool(name="work", bufs=2) as wrk:

        vg3h = [None] * HPC

        cos_s = cpool.tile([128, S], DT, name="cos_s")
        srot_s = cpool.tile([128, S], DT, name="srot_s")
        mask_s = cpool.tile([128, 4, SBLK], DT, name="mask_s")
        # bf16 ones matrix: ones.T @ x sums x over partitions, result
        # replicated across partitions (softmax denominator pre-broadcast).
        ones_b = cpool.tile([128, 128], DT, name="ones_b")
        nc.vector.memset(ones_b[:], 1.0)
        # Tiny warm-up AllGather issued first: the first collective of an
        # execution pays ~100us of one-time + entry-barrier overhead; absorb
        # it under the projection phase so the real AG chain starts early.
        nc.gpsimd.dma_start(wup_in[:, :], ones_b[:, 0:4])
        nc.gpsimd.collective_compute(
            "AllGather", mybir.AluOpType.bypass,
            replica_groups=[[0, 1, 2, 3], [4, 5, 6, 7]],
            ins=[wup_in[:, :].opt()], outs=[wup_out[:, :].opt()],
        )

        qT = qkvpool.tile([128, HPC, S], DT, name="qT")
        kTt = qkvpool.tile([128, HPC, S], DT, name="kTt")
        v_s = qkvpool.tile([128, KT, HPC * DH], DT, name="v_s")

        # Weight pool: 3 rotating slots -- wo reuses wq's slot once the
        # second superblock's Q sweeps have drained it.
        wq_s = wpool.tile([128, DTILES, HPC * DH], DT, tag="w", name="wq_s",
                          bufs=3)
        wk_s = wpool.tile([128, DTILES, HPC * DH], DT, tag="w", name="wk_s",
                          bufs=3)
        wv_s = wpool.tile([128, DTILES, HPC * DH], DT, tag="w", name="wv_s",
                          bufs=3)

        # x superblock tiles [128, dt, 2 blocks, 512]: the Q/K sweeps stream
        # both seq blocks as a single 1024-wide bf16 moving operand (the HW
        # moving-dim limit for bf16 is 1024, not 512).
        x_tiles = [None, None]
        x_tiles[0] = xpool.tile([128, DTILES, 2, SBLK], DT, tag="x2",
                                name="x01")
        fine = 8
        fstep = DTILES // fine

        def _x2_load(dst, sb0, i):
            t0 = i * fstep
            nc.sync.dma_start(
                dst[:, t0:t0 + fstep, :, :],
                xT[t0 * 128:(t0 + fstep) * 128,
                   sb0 * SBLK:(sb0 + 2) * SBLK].rearrange(
                    "(t p) (j s) -> p t j s", p=128, j=2))

        def _fchunk(dst, src_2d, i):
            t0 = i * fstep
            nc.sync.dma_start(
                dst[:, t0:t0 + fstep, :],
                src_2d[t0 * 128:(t0 + fstep) * 128, :].rearrange(
                    "(t p) s -> p t s", p=128))

        for i in range(fine):
            _fchunk(wq_s, wqT, i)
            _x2_load(x_tiles[0], 0, i)
        nc.sync.dma_start(cos_s[:], cosT[:, :])
        nc.sync.dma_start(srot_s[:], srotT[:, :])
        _split_load(nc, wk_s, wkT)
        nc.sync.dma_start(mask_s[:], masks.rearrange("m p s -> p m s"))
        _split_load(nc, wv_s, wvT)

        def qk_sweep(h, w_s, dstT, blkA, blkB, x2):
            """Project head h for both seq blocks of a superblock as one
            sweep of 1024-wide matmuls, then apply RoPE to both halves."""
            ps = psp.tile([128, 2, SBLK], F32, tag="main", name="ps_qk")
            for dt_i in range(DTILES):
                for half in range(2):
                    nc.tensor.matmul(
                        ps[:, half, :],
                        lhsT=w_s[:, dt_i, h * DH:(h + 1) * DH],
                        rhs=x2[:, dt_i, half, :],
                        start=(dt_i == 0), stop=(dt_i == DTILES - 1),
                    )
            raw = wrk.tile([128, 2, SBLK], DT, tag="raw", name="raw")
            nc.scalar.copy(raw[:], ps[:])
            # rotate-half along partitions: engines can't shift partitions,
            # DMA can.
            rot = wrk.tile([128, 2, SBLK], DT, tag="rot", name="rot")
            nc.scalar.dma_start(rot[0:64], raw[64:128])
            nc.scalar.dma_start(rot[64:128], raw[0:64])
            for half, blk in ((0, blkA), (1, blkB)):
                s0 = blk * SBLK
                nc.vector.tensor_mul(rot[:, half, :], rot[:, half, :],
                                     srot_s[:, s0:s0 + SBLK])
                nc.vector.tensor_mul(raw[:, half, :], raw[:, half, :],
                                     cos_s[:, s0:s0 + SBLK])
                nc.vector.tensor_add(dstT[:, h, s0:s0 + SBLK],
                                     raw[:, half, :], rot[:, half, :])

        def v_pair_sweep(sb, hp, x2, half):
            """V projection for seq sub-tiles (2hp, 2hp+1) of block sb."""
            ps = psp.tile([128, 2, SBLK], F32, tag="main", name="ps_v")
            for dt_i in range(DTILES):
                st_ = dt_i == 0
                sp_ = dt_i == DTILES - 1
                for st in range(2):
                    t = 2 * hp + st
                    nc.tensor.matmul(
                        ps[:, st, :],
                        lhsT=x2[:, dt_i, half, t * 128:(t + 1) * 128],
                        rhs=wv_s[:, dt_i, :],
                        start=st_, stop=sp_,
                    )
            nc.scalar.copy(v_s[:, sb * 4 + 2 * hp:sb * 4 + 2 * hp + 2, :],
                           ps[:])

        def attn_block(sqb):
            """Causal attention for query block sqb, all 4 heads.

            Software-pipelined one k-tile-pair ahead: pair j+1's scores and
            exp are emitted before pair j's AV matmuls, so TensorE never
            waits the exp latency.  For the last block, each head's output
            is AllGather'd as soon as the head completes."""
            sq0 = sqb * SBLK
            nkt = 4 * (sqb + 1)
            npair = nkt // 2
            for h in range(HPC):
                ps_av = psp.tile([128, SBLK], F32, tag="av", name="ps_av")
                ps_den = psp.tile([128, SBLK], F32, tag="den", name="ps_den",
                                  bufs=1)
                qsum = None
                pending = None  # (st_e, pair_idx) awaiting AV emission

                def emit_scores(j):
                    ps_st = psp.tile([128, 2, SBLK], F32, tag="main",
                                     name="ps_st")
                    for i in range(2):
                        kt = 2 * j + i
                        nc.tensor.matmul(
                            ps_st[:, i, :],
                            lhsT=kTt[:, h, kt * 128:(kt + 1) * 128],
                            rhs=qT[:, h, sq0:sq0 + SBLK],
                            start=True, stop=True,
                        )
                    st_e = wrk.tile([128, 2, SBLK], DT, tag="ste",
                                    name="st_e", bufs=2)
                    nc.scalar.activation(st_e[:], ps_st[:], AF.Exp,
                                         scale=SCALE)
                    p = 2 * j - (nkt - 4)
                    if p >= 0:  # diagonal pair: causal 0/1 mask
                        nc.vector.tensor_mul(st_e[:], st_e[:],
                                             mask_s[:, p:p + 2, :])
                    return st_e

                def emit_av(st_e, j):
                    nonlocal qsum
                    for i in range(2):
                        kt = 2 * j + i
                        nc.tensor.matmul(
                            ps_av[:],
                            lhsT=v_s[:, kt, h * DH:(h + 1) * DH],
                            rhs=st_e[:, i, :],
                            start=(kt == 0), stop=(kt == nkt - 1),
                        )
                    # quad-sum of exp tiles for the denominator: 3 DVE adds
                    # per 4 k-tiles, one ones-matmul per quad.
                    if j % 2 == 0:
                        qsum = wrk.tile([128, SBLK], DT, tag="qsum",
                                        name="qsum", bufs=1)
                        nc.vector.tensor_add(qsum[:], st_e[:, 0, :],
                                             st_e[:, 1, :])
                    else:
                        nc.vector.tensor_add(qsum[:], qsum[:], st_e[:, 0, :])
                        nc.vector.tensor_add(qsum[:], qsum[:], st_e[:, 1, :])
                        q = j // 2
                        nc.tensor.matmul(
                            ps_den[:],
                            lhsT=ones_b[:],
                            rhs=qsum[:],
                            start=(q == 0), stop=(q == npair // 2 - 1),
                        )

                for j in range(npair):
                    st_e = emit_scores(j)
                    if pending is not None:
                        emit_av(*pending)
                    pending = (st_e, j)
                emit_av(*pending)

                rden = wrk.tile([128, SBLK], F32, tag="rden", name="rden",
                                bufs=1)
                nc.vector.reciprocal_approx_fast(rden[:], ps_den[:])
                vout = wrk.tile([128, SBLK], DT, tag="vout", name="vout",
                                bufs=1)
                nc.vector.tensor_mul(vout[:], ps_av[:], rden[:])
                if sqb == NSB - 1:
                    nc.sync.dma_start(vloc3h[h][:, :], vout[:])
                    # Gather this head immediately, then stage it for the
                    # partial o_proj; only the final head's small AG (and 4
                    # dt-tiles of o_proj) remain in the tail.
                    nc.gpsimd.collective_compute(
                        "AllGather", mybir.AluOpType.bypass,
                        replica_groups=[[0, 1, 2, 3], [4, 5, 6, 7]],
                        ins=[vloc3h[h][:, :].opt()],
                        outs=[vgath3h[h][:, :].opt()],
                    )
                    vg3h[h] = v3pool.tile([128, TP, SBLK], DT, tag="vg3",
                                          name="vg3h", bufs=3)
                    nc.gpsimd.dma_start(
                        vg3h[h][:, :, :],
                        vgath3h[h].rearrange("(r p) s -> p r s", p=128))
                else:
                    nc.sync.dma_start(vloc[sqb][h * DH:(h + 1) * DH, :],
                                      vout[:])

        def ag_block(c):
            nc.gpsimd.collective_compute(
                "AllGather",
                mybir.AluOpType.bypass,
                replica_groups=[[0, 1, 2, 3], [4, 5, 6, 7]],
                ins=[vloc[c][:, :].opt()],
                outs=[vgath[c][:, :].opt()],
            )

        def vg_half_load(vg, half, c):
            """Load gathered chunk c into one half of an x2-shaped tile;
            gpsimd queue so its tile-slot wait sits behind the AllGathers,
            never blocking compute queues."""
            for i_ in range(2):
                t0 = i_ * (DTILES // 2)
                nc.gpsimd.dma_start(
                    vg[:, t0:t0 + DTILES // 2, half, :],
                    vgath[c][t0 * 128:(t0 + DTILES // 2) * 128, :]
                    .rearrange("(t p) s -> p t s", p=128))

        def oproj_pair(cA, vg01):
            """o_proj for chunks (cA, cA+1) as one sweep of 1024-wide
            matmuls over both gathered chunks."""
            for ct in range(OCOLS // 128):
                ps = psp.tile([128, 2, SBLK], F32, tag="main", name="ps_o2")
                for dt_i in range(DTILES):
                    for half in range(2):
                        nc.tensor.matmul(
                            ps[:, half, :],
                            lhsT=wo_s[:, dt_i, ct * 128:(ct + 1) * 128],
                            rhs=vg01[:, dt_i, half, :],
                            start=(dt_i == 0), stop=(dt_i == DTILES - 1),
                        )
                for half in range(2):
                    ob = wrk.tile([128, SBLK], F32, tag="ob", name="ob")
                    nc.scalar.copy(ob[:], ps[:, half, :])
                    nc.scalar.dma_start(
                        outT[ct * 128:(ct + 1) * 128,
                             (cA + half) * SBLK:(cA + half + 1) * SBLK],
                        ob[:])

        def oproj_one(c, vg, half):
            for ct in range(OCOLS // 128):
                ps_o = psp.tile([128, SBLK], F32, tag="av", name="ps_o")
                for dt_i in range(DTILES):
                    nc.tensor.matmul(
                        ps_o[:],
                        lhsT=wo_s[:, dt_i, ct * 128:(ct + 1) * 128],
                        rhs=vg[:, dt_i, half, :],
                        start=(dt_i == 0), stop=(dt_i == DTILES - 1),
                    )
                ob = wrk.tile([128, SBLK], F32, tag="ob", name="ob")
                nc.scalar.copy(ob[:], ps_o[:])
                nc.scalar.dma_start(
                    outT[ct * 128:(ct + 1) * 128, c * SBLK:(c + 1) * SBLK],
                    ob[:])

        def oproj3_chunks(hhs, ps_pair):
            """o_proj partial accumulation for the last seq chunk: head-chunks
            `hhs` of the per-head AllGathers.  vgath3h[h] rows are (rank r,
            partition p) = global input dim r*512 + h*128 + p, i.e. d-tile
            r*4 + h.  All 4 output col-tiles accumulate at once (two
            [128,2,512] psum tiles held open) so each head-chunk is consumed
            the moment it arrives."""
            psA, psB = ps_pair
            for hh in hhs:
                for ct in range(OCOLS // 128):
                    ps, half = (psA, ct) if ct < 2 else (psB, ct - 2)
                    for r in range(TP):
                        nc.tensor.matmul(
                            ps[:, half, :],
                            lhsT=wo_s[:, r * HPC + hh,
                                      ct * 128:(ct + 1) * 128],
                            rhs=vg3h[hh][:, r, :],
                            start=(hh == 0 and r == 0),
                            stop=(hh == HPC - 1 and r == TP - 1),
                        )

        def oproj3_out(ps_pair):
            c3 = NSB - 1
            psA, psB = ps_pair
            for pi, ps in enumerate((psA, psB)):
                for half in range(2):
                    ct = pi * 2 + half
                    ob = wrk.tile([128, SBLK], F32, tag="ob", name="ob")
                    nc.scalar.copy(ob[:], ps[:, half, :])
                    nc.scalar.dma_start(
                        outT[ct * 128:(ct + 1) * 128,
                             c3 * SBLK:(c3 + 1) * SBLK],
                        ob[:])

        # ---- main interleaved emission ----
        wo_s = None
        vg01 = None
        vg2 = None
        for sbp in range(2):
            blkA, blkB = 2 * sbp, 2 * sbp + 1
            if sbp == 0:
                # Prefetch the next superblock's x.
                x_tiles[1] = xpool.tile([128, DTILES, 2, SBLK], DT,
                                        tag="x2", name="x23")
                for i in range(fine):
                    _x2_load(x_tiles[1], 2, i)
            x2 = x_tiles[sbp]
            for h in range(HPC):
                qk_sweep(h, wq_s, qT, blkA, blkB, x2)
            if sbp == 1:
                # wq is fully consumed now: its pool slot takes wo (loaded on
                # the gpsimd queue, whose natural slack absorbs the slot
                # wait without blocking other engines).
                wo_s = wpool.tile([128, DTILES, OCOLS], DT, tag="w",
                                  name="wo_s", bufs=3)
                _split_load(nc, wo_s, woT, eng=nc.gpsimd)
            for h in range(HPC):
                qk_sweep(h, wk_s, kTt, blkA, blkB, x2)
            for hp in range(2):
                v_pair_sweep(blkA, hp, x2, 0)
            attn_block(blkA)
            ag_block(blkA)
            for hp in range(2):
                v_pair_sweep(blkB, hp, x2, 1)
            # vg tiles reuse x2 slots; they are allocated only after every
            # reader of the slot's previous occupant has been emitted (the
            # V sweeps above are the last x2 readers of this superblock).
            if sbp == 0:
                attn_block(1)
                ag_block(1)
                vg01 = xpool.tile([128, DTILES, 2, SBLK], DT, tag="x2",
                                  name="vg01")
                vg_half_load(vg01, 0, 0)
                vg_half_load(vg01, 1, 1)
            else:
                vg2 = xpool.tile([128, DTILES, 2, SBLK], DT, tag="x2",
                                 name="vg2")
                vg_half_load(vg2, 0, 2)
                attn_block(3)   # per-head AGs + loads inside

        # Emission order: the scheduler hoists these matmuls into attention's
        # ACT-bound slack as their gathered operands land; oproj2 is
        # sandwiched between oproj3's first three head-chunks and its last
        # one so its matmuls keep the PE busy (and the HAM clock warm) while
        # the final head's AllGather completes.
        oproj_pair(0, vg01)
        ps_pair = (psp.tile([128, 2, SBLK], F32, tag="main", name="ps_o3a"),
                   psp.tile([128, 2, SBLK], F32, tag="main", name="ps_o3b"))
        oproj3_chunks(range(HPC - 1), ps_pair)
        oproj_one(2, vg2, 0)
        oproj3_chunks([HPC - 1], ps_pair)
        oproj3_out(ps_pair)


_NC_CACHE = None


def build_program():
    global _NC_CACHE
    if _NC_CACHE is not None:
        return _NC_CACHE
    nc = bacc.Bacc("TRN2", target_bir_lowering=False, debug=False,
                   enable_asserts=False, num_devices=NCORES)
    with tile.TileContext(nc) as tc:
        _emit(tc)
    nc.compile()
    _NC_CACHE = nc
    return nc


def _prep_inputs(x, cos, sin, Wq, Wk, Wv, Wo):
    """Build the 8 per-core input maps (host-side sharding only)."""
    x = np.asarray(x, dtype=np.float32)
    cos = np.asarray(cos, dtype=np.float32)
    sin = np.asarray(sin, dtype=np.float32)
    Wq = np.asarray(Wq, dtype=np.float32)
    Wk = np.asarray(Wk, dtype=np.float32)
    Wv = np.asarray(Wv, dtype=np.float32)
    Wo = np.asarray(Wo, dtype=np.float32)

    cosT = np.ascontiguousarray(cos.T).astype(NPDT)             # [128, S]
    sinT = np.ascontiguousarray(sin.T)
    srotT = np.concatenate([-sinT[:64], sinT[64:]], axis=0).astype(NPDT)

    iota = np.arange(SBLK)[None, :]
    rows = np.arange(128)[:, None]
    masks = np.stack(
        [(128 * p + rows <= iota) for p in range(4)]).astype(NPDT)  # [4,128,512]

    xTg = [np.ascontiguousarray(x[g].T).astype(NPDT) for g in range(B)]

    in_maps = []
    for c in range(NCORES):
        g, r = c // TP, c % TP
        hs = slice(r * HPC * DH, (r + 1) * HPC * DH)
        in_maps.append({
            "xT": xTg[g],
            "wqT": np.ascontiguousarray(Wq[hs].T).astype(NPDT),
            "wkT": np.ascontiguousarray(Wk[hs].T).astype(NPDT),
            "wvT": np.ascontiguousarray(Wv[hs].T).astype(NPDT),
            "woT": np.ascontiguousarray(Wo[r * OCOLS:(r + 1) * OCOLS].T).astype(NPDT),
            "cosT": cosT,
            "srotT": srotT,
            "masks": masks,
        })
    return in_maps


def run(inputs, trace=False, trace_cores=None):
    nc = build_program()
    in_maps = _prep_inputs(**inputs)
    res = run_bass_kernel_spmd(
        nc, in_maps, core_ids=list(range(NCORES)),
        trace=trace, trace_cores=trace_cores,
    )
    out = np.empty((B, S, D), dtype=np.float32)
    for c in range(NCORES):
        g, r = c // TP, c % TP
        out[g, :, r * OCOLS:(r + 1) * OCOLS] = res.results[c]["outT"].T
    return out, res


def kernel(**inputs):
    out, _ = run(inputs)
    return out


# revision 28
# speedup vs baseline: 1.0330x; 1.0330x over previous
"""Llama attention layer (B=2, S=2048, D=2048, H=16, DH=128) on 8 TRN2 NeuronCores.

Sharding: 2-way data parallel over batch x 4-way tensor parallel over heads.
Core c: batch g = c // 4, heads 4r..4r+3 where r = c % 4.
Projections are column-parallel (each core computes Q/K/V for its 4 heads),
attention is fully local per (batch, head), then the per-head attention
outputs (kept transposed, [dim, seq]) are AllGather'd within each 4-core
batch group in seq-chunks, and o_proj is column-parallel: core c computes
output columns r*512..(r+1)*512 of its batch. Host concatenates.

v3 structure:
- Projection and attention blocks are emitted interleaved in two
  superblocks (P01 A0 AG0 A1 AG1 | P23 A2 AG2 A3 AG3) so each seq-chunk's
  AllGather triggers as soon as that query block's attention finishes; the
  serial AG chain runs entirely under the remaining compute.
- Q/K sweeps process one head over BOTH seq blocks of a superblock with
  the weight tile stationary, so consecutive matmuls share one LDWEIGHTS.
  o_proj chunks (0,1) are paired the same way over one weight sweep.
- The LAST block's AllGather is split per head: each head's output is
  gathered as soon as that head finishes, so only the final head's small
  AG sits in the tail.
- PSUM budget 7 banks: [128,2,512] "main" x2 (all sweeps + score tiles),
  [128,512] "av" x2, [128,512] "den" x1.
- Attention is software-pipelined one k-tile-pair ahead (scores+exp of
  pair j+1 issue before the AV matmuls of pair j) so the ACT engine's exp
  latency stays off the TensorE critical path.

All matmul operands are bf16 (fp32 accumulation in PSUM); softmax runs
without max-subtraction; the denominator is accumulated with ones-matmuls
(partition-replicated).
"""

import os
import sys

for _p in ("/opt/trn_rl_repo", "/root/.axon_site/_ro/trn_rl_repo"):
    if os.path.isdir(_p) and _p not in sys.path:
        sys.path.append(_p)

import numpy as np
import ml_dtypes

import concourse.bass as bass
import concourse.tile as tile
import concourse.mybir as mybir
from concourse import bacc
from concourse.bass_utils import run_bass_kernel_spmd

F32 = mybir.dt.float32
BF16 = mybir.dt.bfloat16
AF = mybir.ActivationFunctionType

B, S, D, H, DH = 2, 2048, 2048, 16, 128
NCORES = 8
TP = 4                 # cores per batch group
HPC = H // TP          # heads per core = 4
SBLK = 512             # seq block (matmul moving size)
NSB = S // SBLK        # 4
DTILES = D // 128      # 16 contraction tiles
KT = S // 128          # 16 key tiles
OCOLS = D // TP        # 512 output columns per core
SCALE = 1.0 / float(np.sqrt(DH))

DT = BF16              # matmul operand dtype
NPDT = ml_dtypes.bfloat16


def _split_load(nc, dst, src_2d, nchunks=4, eng=None):
    """DMA a [D, inner] DRAM tensor into dst [128, DTILES, inner] in
    chunks along the d-tile axis."""
    eng = eng or nc.sync
    step = DTILES // nchunks
    for i in range(nchunks):
        t0 = i * step
        eng.dma_start(
            dst[:, t0:t0 + step, :],
            src_2d[t0 * 128:(t0 + step) * 128, :].rearrange(
                "(t p) s -> p t s", p=128),
        )


def _emit(tc):
    nc = tc.nc
    xT = nc.dram_tensor("xT", [D, S], DT, kind="ExternalInput").ap()
    wqT = nc.dram_tensor("wqT", [D, HPC * DH], DT, kind="ExternalInput").ap()
    wkT = nc.dram_tensor("wkT", [D, HPC * DH], DT, kind="ExternalInput").ap()
    wvT = nc.dram_tensor("wvT", [D, HPC * DH], DT, kind="ExternalInput").ap()
    woT = nc.dram_tensor("woT", [D, OCOLS], DT, kind="ExternalInput").ap()
    cosT = nc.dram_tensor("cosT", [DH, S], DT, kind="ExternalInput").ap()
    srotT = nc.dram_tensor("srotT", [DH, S], DT, kind="ExternalInput").ap()
    masks = nc.dram_tensor("masks", [4, 128, SBLK], DT, kind="ExternalInput").ap()
    outT = nc.dram_tensor("outT", [OCOLS, S], F32, kind="ExternalOutput").ap()

    # AllGather bounce buffers: whole-block chunks for blocks 0..2, per-head
    # chunks for the last block (so the tail only waits on one head's AG).
    vloc = [nc.dram_tensor(f"vals_loc_{c}", [HPC * DH, SBLK], DT).ap()
            for c in range(NSB - 1)]
    vgath = [nc.dram_tensor(f"vals_gath_{c}", [D, SBLK], DT).ap()
             for c in range(NSB - 1)]
    vloc3h = [nc.dram_tensor(f"vals_loc3_{h}", [DH, SBLK], DT).ap()
              for h in range(HPC)]
    vgath3h = [nc.dram_tensor(f"vals_gath3_{h}", [TP * DH, SBLK], DT).ap()
               for h in range(HPC)]
    wup_in = nc.dram_tensor("wup_in", [128, 4], DT).ap()
    wup_out = nc.dram_tensor("wup_out", [512, 4], DT).ap()

    with tc.tile_pool(name="const", bufs=1) as cpool, \
         tc.tile_pool(name="qkv", bufs=1) as qkvpool, \
         tc.tile_pool(name="wts", bufs=1) as wpool, \
         tc.tile_pool(name="xvg", bufs=3) as xpool, \
         tc.tile_pool(name="v3p", bufs=3) as v3pool, \
         tc.tile_pool(name="ps", bufs=2, space="PSUM") as psp, \
         tc.tile_pool(name="work", bufs=2) as wrk:

        vg3h = [None] * HPC

        cos_s = cpool.tile([128, S], DT, name="cos_s")
        srot_s = cpool.tile([128, S], DT, name="srot_s")
        mask_s = cpool.tile([128, 4, SBLK], DT, name="mask_s")
        # bf16 ones matrix: ones.T @ x sums x over partitions, result
        # replicated across partitions (softmax denominator pre-broadcast).
        ones_b = cpool.tile([128, 128], DT, name="ones_b")
        nc.vector.memset(ones_b[:], 1.0)
        # Tiny warm-up AllGather issued first: the first collective of an
        # execution pays ~100us of one-time + entry-barrier overhead; absorb
        # it under the projection phase so the real AG chain starts early.
        nc.gpsimd.dma_start(wup_in[:, :], ones_b[:, 0:4])
        nc.gpsimd.collective_compute(
            "AllGather", mybir.AluOpType.bypass,
            replica_groups=[[0, 1, 2, 3], [4, 5, 6, 7]],
            ins=[wup_in[:, :].opt()], outs=[wup_out[:, :].opt()],
        )

        qT = qkvpool.tile([128, HPC, S], DT, name="qT")
        kTt = qkvpool.tile([128, HPC, S], DT, name="kTt")
        v_s = qkvpool.tile([128, KT, HPC * DH], DT, name="v_s")

        wq_s = wpool.tile([128, DTILES, HPC * DH], DT, name="wq_s")
        wk_s = wpool.tile([128, DTILES, HPC * DH], DT, name="wk_s")
        wv_s = wpool.tile([128, DTILES, HPC * DH], DT, name="wv_s")
        wo_s = wpool.tile([128, DTILES, OCOLS], DT, name="wo_s")

        # Startup loads: the first Q sweep streams all of wq plus x blocks
        # 0 AND 1 (superblock pairing), so those three lead, finely chunked.
        x_tiles = [None] * NSB
        for sb in range(2):
            x_tiles[sb] = xpool.tile([128, DTILES, SBLK], DT, tag="x",
                                     name="x_s")
        fine = 8
        fstep = DTILES // fine

        def _fchunk(dst, src_2d, i):
            t0 = i * fstep
            nc.sync.dma_start(
                dst[:, t0:t0 + fstep, :],
                src_2d[t0 * 128:(t0 + fstep) * 128, :].rearrange(
                    "(t p) s -> p t s", p=128))

        for i in range(fine):
            _fchunk(wq_s, wqT, i)
            _fchunk(x_tiles[0], xT[:, 0:SBLK], i)
            _fchunk(x_tiles[1], xT[:, SBLK:2 * SBLK], i)
        nc.sync.dma_start(cos_s[:], cosT[:, :])
        nc.sync.dma_start(srot_s[:], srotT[:, :])
        _split_load(nc, wk_s, wkT)
        nc.sync.dma_start(mask_s[:], masks.rearrange("m p s -> p m s"))
        _split_load(nc, wv_s, wvT)
        _split_load(nc, wo_s, woT)

        def qk_sweep(h, w_s, dstT, blkA, blkB, xA, xB):
            """Project head h for seq blocks (blkA, blkB) with the weight
            tile stationary across both (one LDWEIGHTS per two matmuls),
            then apply RoPE to both halves."""
            ps = psp.tile([128, 2, SBLK], F32, tag="main", name="ps_qk")
            for dt_i in range(DTILES):
                st_ = dt_i == 0
                sp_ = dt_i == DTILES - 1
                for half, x_s in ((0, xA), (1, xB)):
                    nc.tensor.matmul(
                        ps[:, half, :],
                        lhsT=w_s[:, dt_i, h * DH:(h + 1) * DH],
                        rhs=x_s[:, dt_i, :],
                        start=st_, stop=sp_,
                    )
            raw = wrk.tile([128, 2, SBLK], DT, tag="raw", name="raw")
            nc.scalar.copy(raw[:], ps[:])
            # rotate-half along partitions: engines can't shift partitions,
            # DMA can.
            rot = wrk.tile([128, 2, SBLK], DT, tag="rot", name="rot")
            nc.scalar.dma_start(rot[0:64], raw[64:128])
            nc.scalar.dma_start(rot[64:128], raw[0:64])
            for half, blk in ((0, blkA), (1, blkB)):
                s0 = blk * SBLK
                nc.vector.tensor_mul(rot[:, half, :], rot[:, half, :],
                                     srot_s[:, s0:s0 + SBLK])
                nc.vector.tensor_mul(raw[:, half, :], raw[:, half, :],
                                     cos_s[:, s0:s0 + SBLK])
                nc.vector.tensor_add(dstT[:, h, s0:s0 + SBLK],
                                     raw[:, half, :], rot[:, half, :])

        def v_pair_sweep(sb, hp, x_s):
            """V projection for seq sub-tiles (2hp, 2hp+1) of block sb."""
            ps = psp.tile([128, 2, SBLK], F32, tag="main", name="ps_v")
            for dt_i in range(DTILES):
                st_ = dt_i == 0
                sp_ = dt_i == DTILES - 1
                for st in range(2):
                    t = 2 * hp + st
                    nc.tensor.matmul(
                        ps[:, st, :],
                        lhsT=x_s[:, dt_i, t * 128:(t + 1) * 128],
                        rhs=wv_s[:, dt_i, :],
                        start=st_, stop=sp_,
                    )
            nc.scalar.copy(v_s[:, sb * 4 + 2 * hp:sb * 4 + 2 * hp + 2, :],
                           ps[:])

        def attn_block(sqb):
            """Causal attention for query block sqb, all 4 heads.

            Software-pipelined one k-tile-pair ahead: pair j+1's scores and
            exp are emitted before pair j's AV matmuls, so TensorE never
            waits the exp latency.  For the last block, each head's output
            is AllGather'd as soon as the head completes."""
            sq0 = sqb * SBLK
            nkt = 4 * (sqb + 1)
            npair = nkt // 2
            for h in range(HPC):
                ps_av = psp.tile([128, SBLK], F32, tag="av", name="ps_av")
                ps_den = psp.tile([128, SBLK], F32, tag="den", name="ps_den",
                                  bufs=1)
                qsum = None
                pending = None  # (st_e, pair_idx) awaiting AV emission

                def emit_scores(j):
                    ps_st = psp.tile([128, 2, SBLK], F32, tag="main",
                                     name="ps_st")
                    for i in range(2):
                        kt = 2 * j + i
                        nc.tensor.matmul(
                            ps_st[:, i, :],
                            lhsT=kTt[:, h, kt * 128:(kt + 1) * 128],
                            rhs=qT[:, h, sq0:sq0 + SBLK],
                            start=True, stop=True,
                        )
                    st_e = wrk.tile([128, 2, SBLK], DT, tag="ste",
                                    name="st_e", bufs=2)
                    nc.scalar.activation(st_e[:], ps_st[:], AF.Exp,
                                         scale=SCALE)
                    p = 2 * j - (nkt - 4)
                    if p >= 0:  # diagonal pair: causal 0/1 mask
                        nc.vector.tensor_mul(st_e[:], st_e[:],
                                             mask_s[:, p:p + 2, :])
                    return st_e

                def emit_av(st_e, j):
                    nonlocal qsum
                    for i in range(2):
                        kt = 2 * j + i
                        nc.tensor.matmul(
                            ps_av[:],
                            lhsT=v_s[:, kt, h * DH:(h + 1) * DH],
                            rhs=st_e[:, i, :],
                            start=(kt == 0), stop=(kt == nkt - 1),
                        )
                    # quad-sum of exp tiles for the denominator: 3 DVE adds
                    # per 4 k-tiles, one ones-matmul per quad.
                    if j % 2 == 0:
                        qsum = wrk.tile([128, SBLK], DT, tag="qsum",
                                        name="qsum", bufs=1)
                        nc.vector.tensor_add(qsum[:], st_e[:, 0, :],
                                             st_e[:, 1, :])
                    else:
                        nc.vector.tensor_add(qsum[:], qsum[:], st_e[:, 0, :])
                        nc.vector.tensor_add(qsum[:], qsum[:], st_e[:, 1, :])
                        q = j // 2
                        nc.tensor.matmul(
                            ps_den[:],
                            lhsT=ones_b[:],
                            rhs=qsum[:],
                            start=(q == 0), stop=(q == npair // 2 - 1),
                        )

                for j in range(npair):
                    st_e = emit_scores(j)
                    if pending is not None:
                        emit_av(*pending)
                    pending = (st_e, j)
                emit_av(*pending)

                rden = wrk.tile([128, SBLK], F32, tag="rden", name="rden",
                                bufs=1)
                nc.vector.reciprocal_approx_fast(rden[:], ps_den[:])
                vout = wrk.tile([128, SBLK], DT, tag="vout", name="vout",
                                bufs=1)
                nc.vector.tensor_mul(vout[:], ps_av[:], rden[:])
                if sqb == NSB - 1:
                    nc.sync.dma_start(vloc3h[h][:, :], vout[:])
                    # Gather this head immediately, then stage it for the
                    # partial o_proj; only the final head's small AG (and 4
                    # dt-tiles of o_proj) remain in the tail.
                    nc.gpsimd.collective_compute(
                        "AllGather", mybir.AluOpType.bypass,
                        replica_groups=[[0, 1, 2, 3], [4, 5, 6, 7]],
                        ins=[vloc3h[h][:, :].opt()],
                        outs=[vgath3h[h][:, :].opt()],
                    )
                    vg3h[h] = v3pool.tile([128, TP, SBLK], DT, tag="vg3",
                                          name="vg3h", bufs=3)
                    nc.gpsimd.dma_start(
                        vg3h[h][:, :, :],
                        vgath3h[h].rearrange("(r p) s -> p r s", p=128))
                else:
                    nc.sync.dma_start(vloc[sqb][h * DH:(h + 1) * DH, :],
                                      vout[:])

        def ag_block(c):
            nc.gpsimd.collective_compute(
                "AllGather",
                mybir.AluOpType.bypass,
                replica_groups=[[0, 1, 2, 3], [4, 5, 6, 7]],
                ins=[vloc[c][:, :].opt()],
                outs=[vgath[c][:, :].opt()],
            )

        def vg_load(c):
            """Load a gathered chunk; gpsimd queue so its tile-slot wait sits
            behind the AllGathers, never blocking compute queues."""
            vg = xpool.tile([128, DTILES, SBLK], DT, tag="x", name="vg")
            for i_ in range(2):
                t0 = i_ * (DTILES // 2)
                nc.gpsimd.dma_start(
                    vg[:, t0:t0 + DTILES // 2, :],
                    vgath[c][t0 * 128:(t0 + DTILES // 2) * 128, :]
                    .rearrange("(t p) s -> p t s", p=128))
            return vg

        def oproj_pair(cA, cB, vgA, vgB):
            """o_proj for two chunks with each weight tile stationary across
            both (one LDWEIGHTS per two matmuls)."""
            for ct in range(OCOLS // 128):
                ps = psp.tile([128, 2, SBLK], F32, tag="main", name="ps_o2")
                for dt_i in range(DTILES):
                    st_ = dt_i == 0
                    sp_ = dt_i == DTILES - 1
                    for half, vg in ((0, vgA), (1, vgB)):
                        nc.tensor.matmul(
                            ps[:, half, :],
                            lhsT=wo_s[:, dt_i, ct * 128:(ct + 1) * 128],
                            rhs=vg[:, dt_i, :],
                            start=st_, stop=sp_,
                        )
                for half, c in ((0, cA), (1, cB)):
                    ob = wrk.tile([128, SBLK], F32, tag="ob", name="ob")
                    nc.scalar.copy(ob[:], ps[:, half, :])
                    nc.scalar.dma_start(
                        outT[ct * 128:(ct + 1) * 128,
                             c * SBLK:(c + 1) * SBLK],
                        ob[:])

        def oproj_one(c, vg):
            for ct in range(OCOLS // 128):
                ps_o = psp.tile([128, SBLK], F32, tag="av", name="ps_o")
                for dt_i in range(DTILES):
                    nc.tensor.matmul(
                        ps_o[:],
                        lhsT=wo_s[:, dt_i, ct * 128:(ct + 1) * 128],
                        rhs=vg[:, dt_i, :],
                        start=(dt_i == 0), stop=(dt_i == DTILES - 1),
                    )
                ob = wrk.tile([128, SBLK], F32, tag="ob", name="ob")
                nc.scalar.copy(ob[:], ps_o[:])
                nc.scalar.dma_start(
                    outT[ct * 128:(ct + 1) * 128, c * SBLK:(c + 1) * SBLK],
                    ob[:])

        def oproj3_chunks(hhs, ps_pair):
            """o_proj partial accumulation for the last seq chunk: head-chunks
            `hhs` of the per-head AllGathers.  vgath3h[h] rows are (rank r,
            partition p) = global input dim r*512 + h*128 + p, i.e. d-tile
            r*4 + h.  All 4 output col-tiles accumulate at once (two
            [128,2,512] psum tiles held open) so each head-chunk is consumed
            the moment it arrives."""
            psA, psB = ps_pair
            for hh in hhs:
                for ct in range(OCOLS // 128):
                    ps, half = (psA, ct) if ct < 2 else (psB, ct - 2)
                    for r in range(TP):
                        nc.tensor.matmul(
                            ps[:, half, :],
                            lhsT=wo_s[:, r * HPC + hh,
                                      ct * 128:(ct + 1) * 128],
                            rhs=vg3h[hh][:, r, :],
                            start=(hh == 0 and r == 0),
                            stop=(hh == HPC - 1 and r == TP - 1),
                        )

        def oproj3_out(ps_pair):
            c3 = NSB - 1
            psA, psB = ps_pair
            for pi, ps in enumerate((psA, psB)):
                for half in range(2):
                    ct = pi * 2 + half
                    ob = wrk.tile([128, SBLK], F32, tag="ob", name="ob")
                    nc.scalar.copy(ob[:], ps[:, half, :])
                    nc.scalar.dma_start(
                        outT[ct * 128:(ct + 1) * 128,
                             c3 * SBLK:(c3 + 1) * SBLK],
                        ob[:])

        # ---- main interleaved emission ----
        vg_tiles = [None] * NSB
        for sbp in range(2):
            blkA, blkB = 2 * sbp, 2 * sbp + 1
            if sbp == 0:
                # Prefetch both blocks of the next superblock.
                for sb in (2, 3):
                    x_tiles[sb] = xpool.tile([128, DTILES, SBLK], DT,
                                             tag="x", name="x_s")
                    _split_load(nc, x_tiles[sb],
                                xT[:, sb * SBLK:(sb + 1) * SBLK])
            xA, xB = x_tiles[blkA], x_tiles[blkB]
            for h in range(HPC):
                qk_sweep(h, wq_s, qT, blkA, blkB, xA, xB)
            for h in range(HPC):
                qk_sweep(h, wk_s, kTt, blkA, blkB, xA, xB)
            for hp in range(2):
                v_pair_sweep(blkA, hp, xA)
            attn_block(blkA)
            if blkA < NSB - 1:
                ag_block(blkA)
                vg_tiles[blkA] = vg_load(blkA)
            for hp in range(2):
                v_pair_sweep(blkB, hp, xB)
            attn_block(blkB)   # for blkB == 3: per-head AGs + loads inside
            if blkB < NSB - 1:
                ag_block(blkB)
                vg_tiles[blkB] = vg_load(blkB)

        # Emission order: the scheduler hoists these matmuls into attention's
        # ACT-bound slack as their gathered operands land; oproj3 (whose last
        # head-chunk arrives latest) goes last.
        oproj_pair(0, 1, vg_tiles[0], vg_tiles[1])
        # oproj2 is sandwiched between oproj3's first three head-chunks and
        # its last one: its matmuls keep the PE busy (and the HAM clock warm)
        # while the final head's AllGather completes.
        ps_pair = (psp.tile([128, 2, SBLK], F32, tag="main", name="ps_o3a"),
                   psp.tile([128, 2, SBLK], F32, tag="main", name="ps_o3b"))
        oproj3_chunks(range(HPC - 1), ps_pair)
        oproj_one(2, vg_tiles[2])
        oproj3_chunks([HPC - 1], ps_pair)
        oproj3_out(ps_pair)


_NC_CACHE = None


def build_program():
    global _NC_CACHE
    if _NC_CACHE is not None:
        return _NC_CACHE
    nc = bacc.Bacc("TRN2", target_bir_lowering=False, debug=False,
                   enable_asserts=False, num_devices=NCORES)
    with tile.TileContext(nc) as tc:
        _emit(tc)
    nc.compile()
    _NC_CACHE = nc
    return nc


def _prep_inputs(x, cos, sin, Wq, Wk, Wv, Wo):
    """Build the 8 per-core input maps (host-side sharding only)."""
    x = np.asarray(x, dtype=np.float32)
    cos = np.asarray(cos, dtype=np.float32)
    sin = np.asarray(sin, dtype=np.float32)
    Wq = np.asarray(Wq, dtype=np.float32)
    Wk = np.asarray(Wk, dtype=np.float32)
    Wv = np.asarray(Wv, dtype=np.float32)
    Wo = np.asarray(Wo, dtype=np.float32)

    cosT = np.ascontiguousarray(cos.T).astype(NPDT)             # [128, S]
    sinT = np.ascontiguousarray(sin.T)
    srotT = np.concatenate([-sinT[:64], sinT[64:]], axis=0).astype(NPDT)

    iota = np.arange(SBLK)[None, :]
    rows = np.arange(128)[:, None]
    masks = np.stack(
        [(128 * p + rows <= iota) for p in range(4)]).astype(NPDT)  # [4,128,512]

    xTg = [np.ascontiguousarray(x[g].T).astype(NPDT) for g in range(B)]

    in_maps = []
    for c in range(NCORES):
        g, r = c // TP, c % TP
        hs = slice(r * HPC * DH, (r + 1) * HPC * DH)
        in_maps.append({
            "xT": xTg[g],
            "wqT": np.ascontiguousarray(Wq[hs].T).astype(NPDT),
            "wkT": np.ascontiguousarray(Wk[hs].T).astype(NPDT),
            "wvT": np.ascontiguousarray(Wv[hs].T).astype(NPDT),
            "woT": np.ascontiguousarray(Wo[r * OCOLS:(r + 1) * OCOLS].T).astype(NPDT),
            "cosT": cosT,
            "srotT": srotT,
            "masks": masks,
        })
    return in_maps


def run(inputs, trace=False, trace_cores=None):
    nc = build_program()
    in_maps = _prep_inputs(**inputs)
    res = run_bass_kernel_spmd(
        nc, in_maps, core_ids=list(range(NCORES)),
        trace=trace, trace_cores=trace_cores,
    )
    out = np.empty((B, S, D), dtype=np.float32)
    for c in range(NCORES):
        g, r = c // TP, c % TP
        out[g, :, r * OCOLS:(r + 1) * OCOLS] = res.results[c]["outT"].T
    return out, res


def kernel(**inputs):
    out, _ = run(inputs)
    return out
